# revision 18
# baseline (speedup 1.0000x reference)
"""Baichuan attention (B=2, S=2048, H=4096, 32 heads x 128) on 8 TRN2 NeuronCores.

Tensor-parallel over heads (4 per core); o_proj row-parallel with the
partial-sum reduction done on host during unshard (fp16 partials).

Per-core pipeline, all in bf16 matmuls (fp32 PSUM accumulate).  All tile
pools are allocated once at top level and persist across phases/batches so
phase boundaries overlap (no pool-scope barriers).

  proj(b):  per 512-col s-chunk: Q/K projections (w stationary, x moving),
            RoPE applied on the drains (ACT copy + partition-swap DMA +
            3 bf16 DVE ops) -> persistent bf16 qT/kT [d,h,s].  V pass is
            h-grouped with a 2-buf wv ring -> v_sb [k, kt, d].
  attn(b):  per 512-q chunk, per head, per 128-k block:
            scores MM (narrowed to unmasked cols) -> causal mask add on
            diagonal blocks -> Exp on ACT (bf16) -> ctx MM accumulate
            (narrowed).  pr tiles are quad-summed (t1/t2 on GpSimd, final
            on DVE) and one den MM per quad (one-hot lhsT -> row h of a
            shared [4,512] PSUM den tile).  Then one reciprocal per
            q-chunk, GpSimd partition-broadcast per head, fused
            normalize-to-bf16 drain.
  oproj:    w_o bf16 resident, ctx-stationary MMs pumped lazily during
            later phases; drains alternate Scalar/Vector; fp16 DMA to
            DRAM out issued from the GpSimd queue.
Host: shards/transposes inputs, sums the 8 row-parallel fp16 partials in
fp32.
"""
import os
import sys

for _p in ("/opt/trn_rl_repo", "/root/.axon_site/_ro/trn_rl_repo"):
    if os.path.isdir(_p) and _p not in sys.path:
        sys.path.insert(0, _p)

from contextlib import ExitStack

import ml_dtypes
import numpy as np

import concourse.bass as bass
import concourse.tile as tile
from concourse import bacc, mybir
from concourse.bass_utils import run_bass_kernel_spmd

F32 = mybir.dt.float32
F16 = mybir.dt.float16
BF16 = mybir.dt.bfloat16

B, S, H = 2, 2048, 4096
NH, HD = 32, 128
NCORES = 8
HPC = NH // NCORES          # heads per core = 4
DPC = HPC * HD              # dims per core = 512
ROPE_BASE = 10000.0

SBLK = 512                  # projection s-chunk
NSB = S // SBLK             # 4 s-chunks per batch
QC = 512                    # attention q-chunk
NQC = S // QC               # 4 q-chunks
NHT = H // 128              # 32 contraction tiles
NKT = S // 128              # 16 k-blocks per sequence
EXPF = mybir.ActivationFunctionType.Exp


def _build():
    nc = bacc.Bacc("TRN2", target_bir_lowering=False, debug=False,
                   num_devices=NCORES)

    xT = nc.dram_tensor("xT", [B, NHT, 128, S], BF16, kind="ExternalInput").ap()
    # wqkT[qk, dt, p, h*128+d] = w_{q|k}^T[128*h + p, 128*dt + d]
    wqkT = nc.dram_tensor("wqkT", [2, HPC, 128, NHT * 128], BF16,
                          kind="ExternalInput").ap()
    # wvT[p, h, d] = w_v^T[128*h + p, d]
    wvT = nc.dram_tensor("wvT", [128, NHT, DPC], BF16,
                         kind="ExternalInput").ap()
    # woT[p, oc, h, o] = w_o^T[128*h + p, 512*oc + o]
    woT = nc.dram_tensor("woT", [128, H // 512, HPC, 512], BF16,
                         kind="ExternalInput").ap()
    cosT = nc.dram_tensor("cosT", [HD, S], BF16, kind="ExternalInput").ap()
    sinTm = nc.dram_tensor("sinTm", [HD, S], BF16, kind="ExternalInput").ap()
    # id16[k, h, j] = 1 if j == h else 0, j<128  (den-matmul one-hot lhsT;
    # full 128-col stationary so the den MM runs at full-M speed)
    id16 = nc.dram_tensor("id16", [128, HPC, 128], BF16,
                          kind="ExternalInput").ap()
    masks = nc.dram_tensor("masks", [128, 128], BF16, kind="ExternalInput").ap()

    out = nc.dram_tensor("out", [B, S, H], F16, kind="ExternalOutput").ap()

    with tile.TileContext(nc) as tc, ExitStack() as top:
        persist = top.enter_context(tc.tile_pool(name="persist", bufs=1))

        # ---- persistent SBUF (per-partition bytes in comments) ----
        cos_sb = persist.tile([HD, S], BF16)                    # 4K
        sin_sb = persist.tile([HD, S], BF16)                    # 4K
        id16_sb = persist.tile([128, HPC, 128], BF16)           # 1K
        mask_sb = persist.tile([128, 128], BF16)                # .25K
        wos_all = persist.tile([128, H // 512, HPC, 512], BF16, tag="wos")  # 32K
        qT = persist.tile([128, HPC, S], BF16, tag="qT")        # 16K
        kT = persist.tile([128, HPC, S], BF16, tag="kT")        # 16K
        v_sb = persist.tile([128, NKT, DPC], BF16, tag="v")     # 16K
        ctx_sb = persist.tile([128, HPC, S], BF16, tag="ctx")   # 16K

        # ---- persistent pools (shared across phases & batches) ----
        xpool = top.enter_context(tc.tile_pool(name="xslab", bufs=32))   # 32K
        wpool = top.enter_context(tc.tile_pool(name="wslab", bufs=3))    # 24+24K
        rpool = top.enter_context(tc.tile_pool(name="rope", bufs=2))     # 8K
        prpool = top.enter_context(tc.tile_pool(name="at_pr", bufs=6))   # 6K
        qpool = top.enter_context(tc.tile_pool(name="at_quad", bufs=2))  # 6K
        smpool = top.enter_context(tc.tile_pool(name="at_sm", bufs=2))   # 10K
        oop = top.enter_context(tc.tile_pool(name="oo", bufs=4))         # 4K

        # ---- PSUM: 8 banks total ----
        psA = top.enter_context(tc.tile_pool(name="psA", bufs=4, space="PSUM"))
        psB = top.enter_context(tc.tile_pool(name="psB", bufs=2, space="PSUM"))
        psC = top.enter_context(tc.tile_pool(name="psC", bufs=1, space="PSUM"))
        psD = top.enter_context(tc.tile_pool(name="psD", bufs=1, space="PSUM"))

        # HAM warmup: ~3.5us of dummy matmuls on a zeroed scratch tile so
        # the PE clock is at 2.4GHz when the first real MMs arrive (~6us).
        warm = ctx_sb[:, 0, 0:512]
        nc.vector.memset(warm, 0)
        wps = psB.tile([128, 512], F32, tag="b", name="warmps")
        for _ in range(8):
            nc.tensor.matmul(wps[:], warm[:, 0:128], warm,
                             start=True, stop=True)

        pend_po = []

        def pump_po(po_pool, n=1, tag="po", qeng=None, alt=None):
            if alt is not None and len(pend_po) > 24:
                n += 1
            for k in range(min(n, len(pend_po))):
                b2, oc, st = pend_po.pop(0)
                pool2, tag2 = (alt if (alt and k % 2 == 1)
                               else (po_pool, tag))
                po = pool2.tile([128, 512], F32, tag=tag2, name="po")
                for h2 in range(HPC):
                    nc.tensor.matmul(
                        po[:],
                        ctx_sb[:, h2, st * 128:(st + 1) * 128],
                        wos_all[:, oc, h2, :],
                        start=(h2 == 0), stop=(h2 == HPC - 1))
                ot = oop.tile([128, 512], F16, tag="ot")
                if st % 2 == 0:
                    nc.scalar.copy(ot[:], po[:])
                else:
                    nc.vector.tensor_copy(ot[:], po[:])
                (qeng or (nc.scalar if st % 2 == 0 else nc.sync)).dma_start(
                    out=out[b2, st * 128:(st + 1) * 128,
                            oc * 512:(oc + 1) * 512],
                    in_=ot[:])

        for b in range(B):
            # ---------------- proj(b) ----------------
            for sb in range(NSB):
                s0 = sb * SBLK
                if b == 0 and sb == 1:
                    nc.scalar.dma_start(out=wos_all[:], in_=woT[:])
                if b == 0 and sb == 0:
                    # first weight tile split across both HWDGE queues so the
                    # first MM group can start ~6us in
                    w0 = wpool.tile([128, NHT, 128], BF16, tag="w")
                    nc.sync.dma_start(out=w0[:, 0:16, :], in_=wqkT[0, 0, :, 0:2048])
                    nc.scalar.dma_start(out=w0[:, 16:32, :], in_=wqkT[0, 0, :, 2048:4096])
                xsl = []
                for h in range(NHT):
                    xs = xpool.tile([128, SBLK], BF16, tag="xs")
                    # weights own the sync queue; x streams on other queues
                    # (each HW DMA queue sustains only ~130 GB/s)
                    if b == 0 and sb == 0:
                        eng = (nc.sync, nc.scalar, nc.gpsimd, nc.gpsimd)[h % 4]
                    else:
                        eng = nc.gpsimd
                    eng.dma_start(out=xs[:], in_=xT[b, h, :, s0:s0 + SBLK])
                    xsl.append(xs)
                if b == 0 and sb == 0:
                    # small constants behind the first x tiles (needed by the
                    # first rope drain at ~16us, land ~20us -- fine)
                    nc.scalar.dma_start(out=cos_sb[:], in_=cosT[:])
                    nc.scalar.dma_start(out=sin_sb[:], in_=sinTm[:])
                    nc.scalar.dma_start(out=id16_sb[:], in_=id16[:])
                    nc.scalar.dma_start(out=mask_sb[:], in_=masks[:])

                # Q and K passes: out [d(head dt), s] with rope on drain
                for qk in range(2):
                    for dt in range(HPC):
                        ps = psA.tile([128, SBLK], F32, tag="a",
                                      name=f"pj{qk}{dt}")
                        if b == 0 and sb == 0 and qk == 0 and dt == 0:
                            w = w0
                        else:
                            w = wpool.tile([128, NHT, 128], BF16, tag="w")
                            weng = nc.scalar if (qk == 1 and dt >= 2) else nc.sync
                            weng.dma_start(out=w[:, 0:16, :],
                                           in_=wqkT[qk, dt, :, 0:2048])
                            weng.dma_start(out=w[:, 16:32, :],
                                           in_=wqkT[qk, dt, :, 2048:4096])
                        for h in range(NHT):
                            nc.tensor.matmul(
                                ps[:], w[:, h, :], xsl[h][:],
                                start=(h == 0), stop=(h == NHT - 1))
                        # rope drain -> (qT|kT)[:, dt, s0:s0+SBLK]
                        dst = (qT if qk == 0 else kT)[:, dt, s0:s0 + SBLK]
                        qsb = rpool.tile([128, SBLK], BF16, tag="qsb")
                        nc.scalar.copy(qsb[:], ps[:])
                        qsw = rpool.tile([128, SBLK], BF16, tag="qsw")
                        nc.scalar.dma_start(out=qsw[0:64, :],
                                            in_=qsb[64:128, :])
                        nc.scalar.dma_start(out=qsw[64:128, :],
                                            in_=qsb[0:64, :])
                        t1 = rpool.tile([128, SBLK], BF16, tag="t1")
                        nc.vector.tensor_mul(t1[:], qsb[:],
                                             cos_sb[:, s0:s0 + SBLK])
                        t2 = rpool.tile([128, SBLK], BF16, tag="t2")
                        nc.vector.tensor_mul(t2[:], qsw[:],
                                             sin_sb[:, s0:s0 + SBLK])
                        nc.vector.tensor_add(dst, t1[:], t2[:])
                        pump_po(psD, qeng=nc.scalar, alt=(psB, "b"))

                # V pass: h-grouped so wv streams through a 2-buf ring.
                # psv[st] accumulate over all h; 4 psA banks held at once.
                psv = [psA.tile([128, DPC], F32, tag="a", name=f"pjv{st}")
                       for st in range(SBLK // 128)]
                for g in range(4):
                    wv = wpool.tile([128, 8, DPC], BF16, tag="wv",
                                     bufs=2)
                    nc.gpsimd.dma_start(out=wv[:],
                                        in_=wvT[:, 8 * g:8 * g + 8, :])
                    for hh in range(8):
                        h = 8 * g + hh
                        for st in range(SBLK // 128):
                            nc.tensor.matmul(
                                psv[st][:],
                                xsl[h][:, st * 128:(st + 1) * 128],
                                wv[:, hh, :],
                                start=(h == 0), stop=(h == NHT - 1))
                    pump_po(psD, qeng=nc.scalar, alt=(psB, "b"))
                for st in range(SBLK // 128):
                    nc.vector.tensor_copy(
                        v_sb[:, (s0 + st * 128) // 128, :], psv[st][:])

            # ---------------- attn(b) ----------------
            for qc in range(NQC):
                q0 = qc * QC
                nkt = 4 * qc + 4
                pden = psC.tile([128, QC], F32, tag="c", name="pden")
                pc = []
                # one pass per head (PSUM: 4 pc + 2 pss + 1 den + 1 po).
                # ctx MMs software-pipelined 2 deep behind the score MMs,
                # hiding the mask+exp latency.  Diagonal blocks narrowed
                # to their unmasked q-columns everywhere (scores, exp,
                # ctx); pr is memset-zeroed below c0 only for the quad
                # sum feeding the den MM.
                for h in range(HPC):
                    pch = psA.tile([128, QC], F32, tag="a", name=f"pc{h}")
                    pc.append(pch)
                    pending = []
                    quad = []
                    nq = 0

                    def flush(pch=pch, qc=qc):
                        kt, prt = pending.pop(0)
                        i = kt - 4 * qc
                        c0 = 128 * i if i > 0 else 0
                        nc.tensor.matmul(
                            pch[:, c0:],
                            v_sb[:, kt, h * HD:(h + 1) * HD],
                            prt[:, c0:],
                            start=(kt == 0), stop=(kt == nkt - 1))

                    for kt in range(nkt):
                        i = kt - 4 * qc
                        c0 = 128 * i if i >= 0 else 0
                        pss = psB.tile([128, QC], F32, tag="b", name="pss")
                        nc.tensor.matmul(
                            pss[:, c0:QC],
                            kT[:, h, kt * 128:(kt + 1) * 128],
                            qT[:, h, q0 + c0:q0 + QC],
                            start=True, stop=True)
                        if len(pending) == 2:
                            flush()
                        pr = prpool.tile([128, QC], BF16, tag="pr",
                                         name="pr")
                        nc.scalar.activation(out=pr[:, c0:QC],
                                             in_=pss[:, c0:QC],
                                             func=EXPF)
                        if i >= 0:
                            # zero the masked upper triangle of the diagonal
                            # block (0/1 bf16 mask, off the pss recycle path)
                            nc.vector.tensor_mul(
                                pr[:, c0:c0 + 128], pr[:, c0:c0 + 128],
                                mask_sb[:])
                        pending.append((kt, pr))
                        if i >= 0:
                            # diagonal block: narrowed per-pr den MM (no
                            # zero-fill or quad tree needed)
                            nc.tensor.matmul(
                                pden[:, c0:],
                                id16_sb[:, h, :],
                                pr[:, c0:],
                                start=(h == 0 and qc == 0 and kt == 0),
                                stop=(h == HPC - 1 and kt == nkt - 1))
                        else:
                            quad.append(pr)
                            if len(quad) == 4:
                                # quad-sum (bf16) then one den MM per quad
                                t1q = qpool.tile([128, QC], BF16, tag="t1q", bufs=1)
                                nc.vector.tensor_add(t1q[:], quad[0][:],
                                                     quad[1][:])
                                t2q = qpool.tile([128, QC], BF16, tag="t2q", bufs=1)
                                nc.vector.tensor_add(t2q[:], quad[2][:],
                                                     quad[3][:])
                                t4q = qpool.tile([128, QC], BF16, tag="t4q")
                                nc.vector.tensor_add(t4q[:], t1q[:], t2q[:])
                                nc.tensor.matmul(
                                    pden[:],
                                    id16_sb[:, h, :],
                                    t4q[:],
                                    start=(h == 0 and nq == 0),
                                    stop=False)
                                nq += 1
                                quad = []
                        pump_po(psD, qeng=nc.sync)
                    while pending:
                        flush()
                    pump_po(psD, qeng=nc.sync)
                rec = smpool.tile([HPC, QC], F32, tag="rec", bufs=1)
                nc.vector.reciprocal_approx_fast(out=rec[:], in_=pden[0:HPC, :])
                for h in range(HPC):
                    if h == 0:
                        src_row = rec[0:1, :]
                    else:
                        rh = smpool.tile([1, QC], F32, tag="rh", bufs=1)
                        nc.scalar.dma_start(out=rh[:], in_=rec[h:h + 1, :])
                        src_row = rh[:]
                    rbc = smpool.tile([128, QC], F32, tag="rbc")
                    nc.gpsimd.partition_broadcast(rbc[:], src_row)
                    nc.vector.tensor_mul(ctx_sb[:, h, q0:q0 + QC],
                                         pc[h][:], rbc[:])
                for oc in range(H // 512):
                    for st in range(4 * qc, 4 * qc + 4):
                        pend_po.append((b, oc, st))

            if b == B - 1:
                pump_po(psA, n=len(pend_po), tag="a")

    nc.compile()
    return nc


_CACHE = {}


def _host_prep(x, w_pack, w_o):
    """Build per-core input maps (sharding + layout prep)."""
    x = np.asarray(x, dtype=np.float32)
    w_pack = np.asarray(w_pack, dtype=np.float32)
    w_o = np.asarray(w_o, dtype=np.float32)

    xT = np.ascontiguousarray(
        x.transpose(0, 2, 1).reshape(B, NHT, 128, S)
        .astype(ml_dtypes.bfloat16))                     # [B, 32, 128, S]

    inv_freq = 1.0 / (ROPE_BASE ** (np.arange(0, HD, 2, dtype=np.float32) / HD))
    t = np.arange(S, dtype=np.float32)
    freqs = np.outer(t, inv_freq)                            # [S, HD/2]
    emb = np.concatenate([freqs, freqs], axis=-1)            # [S, HD]
    cosT = np.ascontiguousarray(
        np.cos(emb).T.astype(ml_dtypes.bfloat16))            # [HD, S]
    sinT = np.sin(emb).T.astype(np.float32)
    sinTm = np.concatenate([-sinT[:HD // 2], sinT[HD // 2:]], axis=0)
    sinTm = np.ascontiguousarray(sinTm.astype(ml_dtypes.bfloat16))

    kk2 = np.arange(128)[:, None]
    qq = np.arange(128)[None, :]
    masks = np.ascontiguousarray(
        np.where(kk2 <= qq, 1.0, 0.0).astype(ml_dtypes.bfloat16))  # [128, 128]

    id16 = np.zeros((128, HPC, 128), dtype=np.float32)
    for h in range(HPC):
        id16[:, h, h] = 1.0
    id16 = np.ascontiguousarray(id16.astype(ml_dtypes.bfloat16))

    scale = float(HD) ** -0.5
    in_maps = []
    for c in range(NCORES):
        r0 = c * DPC
        wq = w_pack[r0:r0 + DPC, :] * scale                  # [512, H]
        wk = w_pack[H + r0:H + r0 + DPC, :]
        wv = w_pack[2 * H + r0:2 * H + r0 + DPC, :]
        # wqkT[qk, dt, p, 128h+d] = w^T[128h+p, 128dt+d]
        wqkT = np.stack([wq.T, wk.T], axis=0)                # [2, H, 512]
        wqkT = wqkT.reshape(2, NHT, 128, HPC, 128)           # [2,h,p,dt,d]
        wqkT = wqkT.transpose(0, 3, 2, 1, 4).reshape(2, HPC, 128, NHT * 128)
        wqkT = np.ascontiguousarray(wqkT.astype(ml_dtypes.bfloat16))
        # wvT[p, h, d] = w_v^T[128h+p, d]
        wvT = wv.T.reshape(NHT, 128, DPC).transpose(1, 0, 2)
        wvT = np.ascontiguousarray(wvT.astype(ml_dtypes.bfloat16))
        # woT[p, oc, h, o] = w_o^T[128h+p, 512oc+o]
        woT = w_o[:, r0:r0 + DPC].T.reshape(HPC, 128, H // 512, 512)
        woT = woT.transpose(1, 2, 0, 3)
        woT = np.ascontiguousarray(woT.astype(ml_dtypes.bfloat16))
        in_maps.append({
            "xT": xT, "wqkT": wqkT, "wvT": wvT, "woT": woT,
            "cosT": cosT, "sinTm": sinTm, "id16": id16,
            "masks": masks,
        })
    return in_maps


def kernel(x, w_pack, w_o, _trace=False, _trace_kwargs=None):
    if "nc" not in _CACHE:
        _CACHE["nc"] = _build()
    nc = _CACHE["nc"]

    in_maps = _host_prep(x, w_pack, w_o)
    res = run_bass_kernel_spmd(nc, in_maps, list(range(NCORES)),
                               trace=_trace, **(_trace_kwargs or {}))
    acc = res.results[0]["out"].astype(np.float32)
    for c in range(1, NCORES):
        acc = acc + res.results[c]["out"].astype(np.float32)
    if _trace:
        kernel.last_results = res
    return acc


# revision 19
# speedup vs baseline: 1.0023x; 1.0023x over previous
"""Baichuan attention (B=2, S=2048, H=4096, 32 heads x 128) on 8 TRN2 NeuronCores.

Tensor-parallel over heads (4 per core); o_proj row-parallel with the
partial-sum reduction done on host during unshard (fp16 partials).

Per-core pipeline, all in bf16 matmuls (fp32 PSUM accumulate).  All tile
pools are allocated once at top level and persist across phases/batches so
phase boundaries overlap (no pool-scope barriers).

  proj(b):  per 512-col s-chunk: Q/K projections (w stationary, x moving),
            RoPE applied on the drains (ACT copy + partition-swap DMA +
            3 bf16 DVE ops) -> persistent bf16 qT/kT [d,h,s].  V pass is
            h-grouped with a 2-buf wv ring -> v_sb [k, kt, d].
  attn(b):  per 512-q chunk, per head, per 128-k block:
            scores MM (narrowed to unmasked cols) -> causal mask add on
            diagonal blocks -> Exp on ACT (bf16) -> ctx MM accumulate
            (narrowed).  pr tiles are quad-summed (t1/t2 on GpSimd, final
            on DVE) and one den MM per quad (one-hot lhsT -> row h of a
            shared [4,512] PSUM den tile).  Then one reciprocal per
            q-chunk, GpSimd partition-broadcast per head, fused
            normalize-to-bf16 drain.
  oproj:    w_o bf16 resident, ctx-stationary MMs pumped lazily during
            later phases; drains alternate Scalar/Vector; fp16 DMA to
            DRAM out issued from the GpSimd queue.
Host: shards/transposes inputs, sums the 8 row-parallel fp16 partials in
fp32.
"""
import os
import sys

for _p in ("/opt/trn_rl_repo", "/root/.axon_site/_ro/trn_rl_repo"):
    if os.path.isdir(_p) and _p not in sys.path:
        sys.path.insert(0, _p)

from contextlib import ExitStack

import ml_dtypes
import numpy as np

import concourse.bass as bass
import concourse.tile as tile
from concourse import bacc, mybir
from concourse.bass_utils import run_bass_kernel_spmd

F32 = mybir.dt.float32
F16 = mybir.dt.float16
BF16 = mybir.dt.bfloat16

B, S, H = 2, 2048, 4096
NH, HD = 32, 128
NCORES = 8
HPC = NH // NCORES          # heads per core = 4
DPC = HPC * HD              # dims per core = 512
ROPE_BASE = 10000.0

SBLK = 512                  # projection s-chunk
NSB = S // SBLK             # 4 s-chunks per batch
QC = 512                    # attention q-chunk
NQC = S // QC               # 4 q-chunks
NHT = H // 128              # 32 contraction tiles
NKT = S // 128              # 16 k-blocks per sequence
EXPF = mybir.ActivationFunctionType.Exp


def _build():
    nc = bacc.Bacc("TRN2", target_bir_lowering=False, debug=False,
                   num_devices=NCORES)

    xT = nc.dram_tensor("xT", [B, NHT, 128, S], BF16, kind="ExternalInput").ap()
    # wqkT[qk, dt, p, h*128+d] = w_{q|k}^T[128*h + p, 128*dt + d]
    wqkT = nc.dram_tensor("wqkT", [2, HPC, 128, NHT * 128], BF16,
                          kind="ExternalInput").ap()
    # wvT[p, h, d] = w_v^T[128*h + p, d]
    wvT = nc.dram_tensor("wvT", [128, NHT, DPC], BF16,
                         kind="ExternalInput").ap()
    # woT[p, oc, h, o] = w_o^T[128*h + p, 512*oc + o]
    woT = nc.dram_tensor("woT", [128, H // 512, HPC, 512], BF16,
                         kind="ExternalInput").ap()
    cosT = nc.dram_tensor("cosT", [HD, S], BF16, kind="ExternalInput").ap()
    sinTm = nc.dram_tensor("sinTm", [HD, S], BF16, kind="ExternalInput").ap()
    # id16[k, h, j] = 1 if j == h else 0, j<128  (den-matmul one-hot lhsT;
    # full 128-col stationary so the den MM runs at full-M speed)
    id16 = nc.dram_tensor("id16", [128, HPC, 128], BF16,
                          kind="ExternalInput").ap()
    masks = nc.dram_tensor("masks", [128, 128], BF16, kind="ExternalInput").ap()

    out = nc.dram_tensor("out", [B, S, H], F16, kind="ExternalOutput").ap()

    with tile.TileContext(nc) as tc, ExitStack() as top:
        persist = top.enter_context(tc.tile_pool(name="persist", bufs=1))

        # ---- persistent SBUF (per-partition bytes in comments) ----
        cos_sb = persist.tile([HD, S], BF16)                    # 4K
        sin_sb = persist.tile([HD, S], BF16)                    # 4K
        id16_sb = persist.tile([128, HPC, 128], BF16)           # 1K
        mask_sb = persist.tile([128, 128], BF16)                # .25K
        wos_all = persist.tile([128, H // 512, HPC, 512], BF16, tag="wos")  # 32K
        qT = persist.tile([128, HPC, S], BF16, tag="qT")        # 16K
        kT = persist.tile([128, HPC, S], BF16, tag="kT")        # 16K
        v_sb = persist.tile([128, NKT, DPC], BF16, tag="v")     # 16K
        ctx_sb = persist.tile([128, HPC, S], BF16, tag="ctx")   # 16K

        # ---- persistent pools (shared across phases & batches) ----
        xpool = top.enter_context(tc.tile_pool(name="xslab", bufs=32))   # 32K
        wpool = top.enter_context(tc.tile_pool(name="wslab", bufs=3))    # 24+24K
        rpool = top.enter_context(tc.tile_pool(name="rope", bufs=2))     # 8K
        prpool = top.enter_context(tc.tile_pool(name="at_pr", bufs=6))   # 6K
        qpool = top.enter_context(tc.tile_pool(name="at_quad", bufs=2))  # 6K
        smpool = top.enter_context(tc.tile_pool(name="at_sm", bufs=2))   # 10K
        oop = top.enter_context(tc.tile_pool(name="oo", bufs=4))         # 4K

        # ---- PSUM: 8 banks total ----
        psA = top.enter_context(tc.tile_pool(name="psA", bufs=4, space="PSUM"))
        psB = top.enter_context(tc.tile_pool(name="psB", bufs=2, space="PSUM"))
        psC = top.enter_context(tc.tile_pool(name="psC", bufs=1, space="PSUM"))
        psD = top.enter_context(tc.tile_pool(name="psD", bufs=1, space="PSUM"))

        # HAM warmup: ~3.5us of dummy matmuls on a zeroed scratch tile so
        # the PE clock is at 2.4GHz when the first real MMs arrive (~6us).
        warm = ctx_sb[:, 0, 0:512]
        nc.vector.memset(warm, 0)
        wps = psB.tile([128, 512], F32, tag="b", name="warmps")
        for _ in range(8):
            nc.tensor.matmul(wps[:], warm[:, 0:128], warm,
                             start=True, stop=True)

        pend_po = []

        def pump_po(po_pool, n=1, tag="po", qeng=None, alt=None):
            if alt is not None and len(pend_po) > 24:
                n += 1
            for k in range(min(n, len(pend_po))):
                b2, oc, st = pend_po.pop(0)
                pool2, tag2 = (alt if (alt and k % 2 == 1)
                               else (po_pool, tag))
                po = pool2.tile([128, 512], F32, tag=tag2, name="po")
                for h2 in range(HPC):
                    nc.tensor.matmul(
                        po[:],
                        ctx_sb[:, h2, st * 128:(st + 1) * 128],
                        wos_all[:, oc, h2, :],
                        start=(h2 == 0), stop=(h2 == HPC - 1))
                ot = oop.tile([128, 512], F16, tag="ot")
                if st % 2 == 0:
                    nc.scalar.copy(ot[:], po[:])
                else:
                    nc.vector.tensor_copy(ot[:], po[:])
                (qeng or (nc.scalar if st % 2 == 0 else nc.sync)).dma_start(
                    out=out[b2, st * 128:(st + 1) * 128,
                            oc * 512:(oc + 1) * 512],
                    in_=ot[:])

        for b in range(B):
            # ---------------- proj(b) ----------------
            for sb in range(NSB):
                s0 = sb * SBLK
                if b == 0 and sb == 1:
                    nc.scalar.dma_start(out=wos_all[:], in_=woT[:])
                if b == 0 and sb == 0:
                    # first weight tile split across both HWDGE queues so the
                    # first MM group can start ~6us in
                    w0 = wpool.tile([128, NHT, 128], BF16, tag="w")
                    nc.sync.dma_start(out=w0[:, 0:16, :], in_=wqkT[0, 0, :, 0:2048])
                    nc.scalar.dma_start(out=w0[:, 16:32, :], in_=wqkT[0, 0, :, 2048:4096])
                xsl = []
                for h in range(NHT):
                    xs = xpool.tile([128, SBLK], BF16, tag="xs")
                    # weights own the sync queue; x streams on other queues
                    # (each HW DMA queue sustains only ~130 GB/s)
                    if b == 0 and sb == 0:
                        eng = (nc.sync, nc.scalar, nc.gpsimd, nc.gpsimd)[h % 4]
                    else:
                        eng = nc.gpsimd
                    eng.dma_start(out=xs[:], in_=xT[b, h, :, s0:s0 + SBLK])
                    xsl.append(xs)
                if b == 0 and sb == 0:
                    # small constants behind the first x tiles (needed by the
                    # first rope drain at ~16us, land ~20us -- fine)
                    nc.scalar.dma_start(out=cos_sb[:], in_=cosT[:])
                    nc.scalar.dma_start(out=sin_sb[:], in_=sinTm[:])
                    nc.scalar.dma_start(out=id16_sb[:], in_=id16[:])
                    nc.scalar.dma_start(out=mask_sb[:], in_=masks[:])

                # Q and K passes: out [d(head dt), s] with rope on drain
                for qk in range(2):
                    for dt in range(HPC):
                        ps = psA.tile([128, SBLK], F32, tag="a",
                                      name=f"pj{qk}{dt}")
                        if b == 0 and sb == 0 and qk == 0 and dt == 0:
                            w = w0
                        else:
                            w = wpool.tile([128, NHT, 128], BF16, tag="w")
                            if b == 0 and sb == 0:
                                weng = (nc.sync, nc.scalar,
                                        nc.gpsimd)[(4 * qk + dt) % 3]
                            else:
                                weng = (nc.scalar if (qk == 1 and dt >= 1)
                                        else nc.sync)
                            weng.dma_start(out=w[:, 0:16, :],
                                           in_=wqkT[qk, dt, :, 0:2048])
                            weng.dma_start(out=w[:, 16:32, :],
                                           in_=wqkT[qk, dt, :, 2048:4096])
                        for h in range(NHT):
                            nc.tensor.matmul(
                                ps[:], w[:, h, :], xsl[h][:],
                                start=(h == 0), stop=(h == NHT - 1))
                        # rope drain -> (qT|kT)[:, dt, s0:s0+SBLK]
                        dst = (qT if qk == 0 else kT)[:, dt, s0:s0 + SBLK]
                        qsb = rpool.tile([128, SBLK], BF16, tag="qsb")
                        nc.scalar.copy(qsb[:], ps[:])
                        qsw = rpool.tile([128, SBLK], BF16, tag="qsw")
                        nc.scalar.dma_start(out=qsw[0:64, :],
                                            in_=qsb[64:128, :])
                        nc.scalar.dma_start(out=qsw[64:128, :],
                                            in_=qsb[0:64, :])
                        t1 = rpool.tile([128, SBLK], BF16, tag="t1")
                        nc.vector.tensor_mul(t1[:], qsb[:],
                                             cos_sb[:, s0:s0 + SBLK])
                        t2 = rpool.tile([128, SBLK], BF16, tag="t2")
                        nc.vector.tensor_mul(t2[:], qsw[:],
                                             sin_sb[:, s0:s0 + SBLK])
                        nc.vector.tensor_add(dst, t1[:], t2[:])
                        pump_po(psD, qeng=nc.scalar, alt=(psB, "b"))

                # V pass: h-grouped so wv streams through a 2-buf ring.
                # psv[st] accumulate over all h; 4 psA banks held at once.
                psv = [psA.tile([128, DPC], F32, tag="a", name=f"pjv{st}")
                       for st in range(SBLK // 128)]
                for g in range(4):
                    wv = wpool.tile([128, 8, DPC], BF16, tag="wv",
                                     bufs=2)
                    nc.gpsimd.dma_start(out=wv[:],
                                        in_=wvT[:, 8 * g:8 * g + 8, :])
                    for hh in range(8):
                        h = 8 * g + hh
                        for st in range(SBLK // 128):
                            nc.tensor.matmul(
                                psv[st][:],
                                xsl[h][:, st * 128:(st + 1) * 128],
                                wv[:, hh, :],
                                start=(h == 0), stop=(h == NHT - 1))
                    pump_po(psD, qeng=nc.scalar, alt=(psB, "b"))
                for st in range(SBLK // 128):
                    nc.vector.tensor_copy(
                        v_sb[:, (s0 + st * 128) // 128, :], psv[st][:])

            # ---------------- attn(b) ----------------
            for qc in range(NQC):
                q0 = qc * QC
                nkt = 4 * qc + 4
                pden = psC.tile([128, QC], F32, tag="c", name="pden")
                pc = []
                # one pass per head (PSUM: 4 pc + 2 pss + 1 den + 1 po).
                # ctx MMs software-pipelined 2 deep behind the score MMs,
                # hiding the mask+exp latency.  Diagonal blocks narrowed
                # to their unmasked q-columns everywhere (scores, exp,
                # ctx); pr is memset-zeroed below c0 only for the quad
                # sum feeding the den MM.
                for h in range(HPC):
                    pch = psA.tile([128, QC], F32, tag="a", name=f"pc{h}")
                    pc.append(pch)
                    pending = []
                    quad = []
                    nq = 0

                    def flush(pch=pch, qc=qc):
                        kt, prt = pending.pop(0)
                        i = kt - 4 * qc
                        c0 = 128 * i if i > 0 else 0
                        nc.tensor.matmul(
                            pch[:, c0:],
                            v_sb[:, kt, h * HD:(h + 1) * HD],
                            prt[:, c0:],
                            start=(kt == 0), stop=(kt == nkt - 1))

                    for kt in range(nkt):
                        i = kt - 4 * qc
                        c0 = 128 * i if i >= 0 else 0
                        pss = psB.tile([128, QC], F32, tag="b", name="pss")
                        nc.tensor.matmul(
                            pss[:, c0:QC],
                            kT[:, h, kt * 128:(kt + 1) * 128],
                            qT[:, h, q0 + c0:q0 + QC],
                            start=True, stop=True)
                        if len(pending) == 2:
                            flush()
                        pr = prpool.tile([128, QC], BF16, tag="pr",
                                         name="pr")
                        nc.scalar.activation(out=pr[:, c0:QC],
                                             in_=pss[:, c0:QC],
                                             func=EXPF)
                        if i >= 0:
                            # zero the masked upper triangle of the diagonal
                            # block (0/1 bf16 mask, off the pss recycle path)
                            nc.vector.tensor_mul(
                                pr[:, c0:c0 + 128], pr[:, c0:c0 + 128],
                                mask_sb[:])
                        pending.append((kt, pr))
                        if i >= 0:
                            # diagonal block: narrowed per-pr den MM (no
                            # zero-fill or quad tree needed)
                            nc.tensor.matmul(
                                pden[:, c0:],
                                id16_sb[:, h, :],
                                pr[:, c0:],
                                start=(h == 0 and qc == 0 and kt == 0),
                                stop=(h == HPC - 1 and kt == nkt - 1))
                        else:
                            quad.append(pr)
                            if len(quad) == 4:
                                # quad-sum (bf16) then one den MM per quad
                                t1q = qpool.tile([128, QC], BF16, tag="t1q", bufs=1)
                                nc.vector.tensor_add(t1q[:], quad[0][:],
                                                     quad[1][:])
                                t2q = qpool.tile([128, QC], BF16, tag="t2q", bufs=1)
                                nc.vector.tensor_add(t2q[:], quad[2][:],
                                                     quad[3][:])
                                t4q = qpool.tile([128, QC], BF16, tag="t4q")
                                nc.vector.tensor_add(t4q[:], t1q[:], t2q[:])
                                nc.tensor.matmul(
                                    pden[:],
                                    id16_sb[:, h, :],
                                    t4q[:],
                                    start=(h == 0 and nq == 0),
                                    stop=False)
                                nq += 1
                                quad = []
                        pump_po(psD, qeng=nc.sync)
                    while pending:
                        flush()
                    pump_po(psD, qeng=nc.sync)
                rec = smpool.tile([HPC, QC], F32, tag="rec", bufs=1)
                nc.vector.reciprocal_approx_fast(out=rec[:], in_=pden[0:HPC, :])
                for h in range(HPC):
                    if h == 0:
                        src_row = rec[0:1, :]
                    else:
                        rh = smpool.tile([1, QC], F32, tag="rh", bufs=1)
                        nc.scalar.dma_start(out=rh[:], in_=rec[h:h + 1, :])
                        src_row = rh[:]
                    rbc = smpool.tile([128, QC], F32, tag="rbc")
                    nc.gpsimd.partition_broadcast(rbc[:], src_row)
                    nc.vector.tensor_mul(ctx_sb[:, h, q0:q0 + QC],
                                         pc[h][:], rbc[:])
                for oc in range(H // 512):
                    for st in range(4 * qc, 4 * qc + 4):
                        pend_po.append((b, oc, st))

            if b == B - 1:
                pump_po(psA, n=len(pend_po), tag="a")

    nc.compile()
    return nc


_CACHE = {}


def _host_prep(x, w_pack, w_o):
    """Build per-core input maps (sharding + layout prep)."""
    x = np.asarray(x, dtype=np.float32)
    w_pack = np.asarray(w_pack, dtype=np.float32)
    w_o = np.asarray(w_o, dtype=np.float32)

    xT = np.ascontiguousarray(
        x.transpose(0, 2, 1).reshape(B, NHT, 128, S)
        .astype(ml_dtypes.bfloat16))                     # [B, 32, 128, S]

    inv_freq = 1.0 / (ROPE_BASE ** (np.arange(0, HD, 2, dtype=np.float32) / HD))
    t = np.arange(S, dtype=np.float32)
    freqs = np.outer(t, inv_freq)                            # [S, HD/2]
    emb = np.concatenate([freqs, freqs], axis=-1)            # [S, HD]
    cosT = np.ascontiguousarray(
        np.cos(emb).T.astype(ml_dtypes.bfloat16))            # [HD, S]
    sinT = np.sin(emb).T.astype(np.float32)
    sinTm = np.concatenate([-sinT[:HD // 2], sinT[HD // 2:]], axis=0)
    sinTm = np.ascontiguousarray(sinTm.astype(ml_dtypes.bfloat16))

    kk2 = np.arange(128)[:, None]
    qq = np.arange(128)[None, :]
    masks = np.ascontiguousarray(
        np.where(kk2 <= qq, 1.0, 0.0).astype(ml_dtypes.bfloat16))  # [128, 128]

    id16 = np.zeros((128, HPC, 128), dtype=np.float32)
    for h in range(HPC):
        id16[:, h, h] = 1.0
    id16 = np.ascontiguousarray(id16.astype(ml_dtypes.bfloat16))

    scale = float(HD) ** -0.5
    in_maps = []
    for c in range(NCORES):
        r0 = c * DPC
        wq = w_pack[r0:r0 + DPC, :] * scale                  # [512, H]
        wk = w_pack[H + r0:H + r0 + DPC, :]
        wv = w_pack[2 * H + r0:2 * H + r0 + DPC, :]
        # wqkT[qk, dt, p, 128h+d] = w^T[128h+p, 128dt+d]
        wqkT = np.stack([wq.T, wk.T], axis=0)                # [2, H, 512]
        wqkT = wqkT.reshape(2, NHT, 128, HPC, 128)           # [2,h,p,dt,d]
        wqkT = wqkT.transpose(0, 3, 2, 1, 4).reshape(2, HPC, 128, NHT * 128)
        wqkT = np.ascontiguousarray(wqkT.astype(ml_dtypes.bfloat16))
        # wvT[p, h, d] = w_v^T[128h+p, d]
        wvT = wv.T.reshape(NHT, 128, DPC).transpose(1, 0, 2)
        wvT = np.ascontiguousarray(wvT.astype(ml_dtypes.bfloat16))
        # woT[p, oc, h, o] = w_o^T[128h+p, 512oc+o]
        woT = w_o[:, r0:r0 + DPC].T.reshape(HPC, 128, H // 512, 512)
        woT = woT.transpose(1, 2, 0, 3)
        woT = np.ascontiguousarray(woT.astype(ml_dtypes.bfloat16))
        in_maps.append({
            "xT": xT, "wqkT": wqkT, "wvT": wvT, "woT": woT,
            "cosT": cosT, "sinTm": sinTm, "id16": id16,
            "masks": masks,
        })
    return in_maps


def kernel(x, w_pack, w_o, _trace=False, _trace_kwargs=None):
    if "nc" not in _CACHE:
        _CACHE["nc"] = _build()
    nc = _CACHE["nc"]

    in_maps = _host_prep(x, w_pack, w_o)
    res = run_bass_kernel_spmd(nc, in_maps, list(range(NCORES)),
                               trace=_trace, **(_trace_kwargs or {}))
    acc = res.results[0]["out"].astype(np.float32)
    for c in range(1, NCORES):
        acc = acc + res.results[c]["out"].astype(np.float32)
    if _trace:
        kernel.last_results = res
    return acc


# revision 20
# speedup vs baseline: 1.0065x; 1.0041x over previous
"""Baichuan attention (B=2, S=2048, H=4096, 32 heads x 128) on 8 TRN2 NeuronCores.

Tensor-parallel over heads (4 per core); o_proj row-parallel with the
partial-sum reduction done on host during unshard (fp16 partials).

Per-core pipeline, all in bf16 matmuls (fp32 PSUM accumulate).  All tile
pools are allocated once at top level and persist across phases/batches so
phase boundaries overlap (no pool-scope barriers).

  proj(b):  per 512-col s-chunk: Q/K projections (w stationary, x moving),
            RoPE applied on the drains (ACT copy + partition-swap DMA +
            3 bf16 DVE ops) -> persistent bf16 qT/kT [d,h,s].  V pass is
            h-grouped with a 2-buf wv ring -> v_sb [k, kt, d].
  attn(b):  per 512-q chunk, per head, per 128-k block:
            scores MM (narrowed to unmasked cols) -> causal mask add on
            diagonal blocks -> Exp on ACT (bf16) -> ctx MM accumulate
            (narrowed).  pr tiles are quad-summed (t1/t2 on GpSimd, final
            on DVE) and one den MM per quad (one-hot lhsT -> row h of a
            shared [4,512] PSUM den tile).  Then one reciprocal per
            q-chunk, GpSimd partition-broadcast per head, fused
            normalize-to-bf16 drain.
  oproj:    w_o bf16 resident, ctx-stationary MMs pumped lazily during
            later phases; drains alternate Scalar/Vector; fp16 DMA to
            DRAM out issued from the GpSimd queue.
Host: shards/transposes inputs, sums the 8 row-parallel fp16 partials in
fp32.
"""
import os
import sys

for _p in ("/opt/trn_rl_repo", "/root/.axon_site/_ro/trn_rl_repo"):
    if os.path.isdir(_p) and _p not in sys.path:
        sys.path.insert(0, _p)

from contextlib import ExitStack

import ml_dtypes
import numpy as np

import concourse.bass as bass
import concourse.tile as tile
from concourse import bacc, mybir
from concourse.bass_utils import run_bass_kernel_spmd

F32 = mybir.dt.float32
F16 = mybir.dt.float16
BF16 = mybir.dt.bfloat16

B, S, H = 2, 2048, 4096
NH, HD = 32, 128
NCORES = 8
HPC = NH // NCORES          # heads per core = 4
DPC = HPC * HD              # dims per core = 512
ROPE_BASE = 10000.0

SBLK = 512                  # projection s-chunk
NSB = S // SBLK             # 4 s-chunks per batch
QC = 512                    # attention q-chunk
NQC = S // QC               # 4 q-chunks
NHT = H // 128              # 32 contraction tiles
NKT = S // 128              # 16 k-blocks per sequence
EXPF = mybir.ActivationFunctionType.Exp


def _build():
    nc = bacc.Bacc("TRN2", target_bir_lowering=False, debug=False,
                   num_devices=NCORES)

    xT = nc.dram_tensor("xT", [B, NHT, 128, S], BF16, kind="ExternalInput").ap()
    # wqkT[qk, dt, p, h*128+d] = w_{q|k}^T[128*h + p, 128*dt + d]
    wqkT = nc.dram_tensor("wqkT", [2, HPC, 128, NHT * 128], BF16,
                          kind="ExternalInput").ap()
    # wvT[p, h, d] = w_v^T[128*h + p, d]
    wvT = nc.dram_tensor("wvT", [128, NHT, DPC], BF16,
                         kind="ExternalInput").ap()
    # woT[p, oc, h, o] = w_o^T[128*h + p, 512*oc + o]
    woT = nc.dram_tensor("woT", [128, H // 512, HPC, 512], BF16,
                         kind="ExternalInput").ap()
    cosT = nc.dram_tensor("cosT", [HD, S], BF16, kind="ExternalInput").ap()
    sinTm = nc.dram_tensor("sinTm", [HD, S], BF16, kind="ExternalInput").ap()
    # id16[k, h, j] = 1 if j == h else 0, j<128  (den-matmul one-hot lhsT;
    # full 128-col stationary so the den MM runs at full-M speed)
    id16 = nc.dram_tensor("id16", [128, HPC, 128], BF16,
                          kind="ExternalInput").ap()
    masks = nc.dram_tensor("masks", [128, 128], BF16, kind="ExternalInput").ap()

    out = nc.dram_tensor("out", [B, S, H], F16, kind="ExternalOutput").ap()

    with tile.TileContext(nc) as tc, ExitStack() as top:
        persist = top.enter_context(tc.tile_pool(name="persist", bufs=1))

        # ---- persistent SBUF (per-partition bytes in comments) ----
        cos_sb = persist.tile([HD, S], BF16)                    # 4K
        sin_sb = persist.tile([HD, S], BF16)                    # 4K
        id16_sb = persist.tile([128, HPC, 128], BF16)           # 1K
        mask_sb = persist.tile([128, 128], BF16)                # .25K
        wos_all = persist.tile([128, H // 512, HPC, 512], BF16, tag="wos")  # 32K
        qT = persist.tile([128, HPC, S], BF16, tag="qT")        # 16K
        kT = persist.tile([128, HPC, S], BF16, tag="kT")        # 16K
        v_sb = persist.tile([128, NKT, DPC], BF16, tag="v")     # 16K
        ctx_sb = persist.tile([128, HPC, S], BF16, tag="ctx")   # 16K

        # ---- persistent pools (shared across phases & batches) ----
        xpool = top.enter_context(tc.tile_pool(name="xslab", bufs=32))   # 32K
        wpool = top.enter_context(tc.tile_pool(name="wslab", bufs=3))    # 24+24K
        rpool = top.enter_context(tc.tile_pool(name="rope", bufs=2))     # 8K
        prpool = top.enter_context(tc.tile_pool(name="at_pr", bufs=7))   # 7K
        qpool = top.enter_context(tc.tile_pool(name="at_quad", bufs=2))  # 6K
        smpool = top.enter_context(tc.tile_pool(name="at_sm", bufs=2))   # 10K
        oop = top.enter_context(tc.tile_pool(name="oo", bufs=4))         # 4K

        # ---- PSUM: 8 banks total ----
        psA = top.enter_context(tc.tile_pool(name="psA", bufs=4, space="PSUM"))
        psB = top.enter_context(tc.tile_pool(name="psB", bufs=2, space="PSUM"))
        psC = top.enter_context(tc.tile_pool(name="psC", bufs=1, space="PSUM"))
        psD = top.enter_context(tc.tile_pool(name="psD", bufs=1, space="PSUM"))

        # HAM warmup: ~3.5us of dummy matmuls on a zeroed scratch tile so
        # the PE clock is at 2.4GHz when the first real MMs arrive (~6us).
        warm = ctx_sb[:, 0, 0:512]
        nc.vector.memset(warm, 0)
        wps = psB.tile([128, 512], F32, tag="b", name="warmps")
        for _ in range(8):
            nc.tensor.matmul(wps[:], warm[:, 0:128], warm,
                             start=True, stop=True)

        pend_po = []

        def pump_po(po_pool, n=1, tag="po", qeng=None, alt=None):
            if alt is not None and len(pend_po) > 24:
                n += 1
            for k in range(min(n, len(pend_po))):
                b2, oc, st = pend_po.pop(0)
                pool2, tag2 = (alt if (alt and k % 2 == 1)
                               else (po_pool, tag))
                po = pool2.tile([128, 512], F32, tag=tag2, name="po")
                for h2 in range(HPC):
                    nc.tensor.matmul(
                        po[:],
                        ctx_sb[:, h2, st * 128:(st + 1) * 128],
                        wos_all[:, oc, h2, :],
                        start=(h2 == 0), stop=(h2 == HPC - 1))
                ot = oop.tile([128, 512], F16, tag="ot")
                if st % 2 == 0:
                    nc.scalar.copy(ot[:], po[:])
                else:
                    nc.vector.tensor_copy(ot[:], po[:])
                (qeng or (nc.scalar if st % 2 == 0 else nc.sync)).dma_start(
                    out=out[b2, st * 128:(st + 1) * 128,
                            oc * 512:(oc + 1) * 512],
                    in_=ot[:])

        for b in range(B):
            # ---------------- proj(b) ----------------
            for sb in range(NSB):
                s0 = sb * SBLK
                if b == 0 and sb == 1:
                    nc.scalar.dma_start(out=wos_all[:], in_=woT[:])
                if b == 0 and sb == 0:
                    # first weight tile split across both HWDGE queues so the
                    # first MM group can start ~6us in
                    w0 = wpool.tile([128, NHT, 128], BF16, tag="w")
                    nc.sync.dma_start(out=w0[:, 0:16, :], in_=wqkT[0, 0, :, 0:2048])
                    nc.scalar.dma_start(out=w0[:, 16:32, :], in_=wqkT[0, 0, :, 2048:4096])
                xsl = []
                for h in range(NHT):
                    xs = xpool.tile([128, SBLK], BF16, tag="xs")
                    # weights own the sync queue; x streams on other queues
                    # (each HW DMA queue sustains only ~130 GB/s)
                    if b == 0 and sb == 0:
                        eng = (nc.sync, nc.scalar, nc.gpsimd, nc.gpsimd)[h % 4]
                    else:
                        eng = nc.gpsimd
                    eng.dma_start(out=xs[:], in_=xT[b, h, :, s0:s0 + SBLK])
                    xsl.append(xs)
                if b == 0 and sb == 0:
                    # small constants behind the first x tiles (needed by the
                    # first rope drain at ~16us, land ~20us -- fine)
                    nc.scalar.dma_start(out=cos_sb[:], in_=cosT[:])
                    nc.scalar.dma_start(out=sin_sb[:], in_=sinTm[:])
                    nc.scalar.dma_start(out=id16_sb[:], in_=id16[:])
                    nc.scalar.dma_start(out=mask_sb[:], in_=masks[:])

                # Q and K passes: out [d(head dt), s] with rope on drain
                for qk in range(2):
                    for dt in range(HPC):
                        ps = psA.tile([128, SBLK], F32, tag="a",
                                      name=f"pj{qk}{dt}")
                        if b == 0 and sb == 0 and qk == 0 and dt == 0:
                            w = w0
                        else:
                            w = wpool.tile([128, NHT, 128], BF16, tag="w")
                            if b == 0 and sb == 0:
                                weng = (nc.sync, nc.scalar,
                                        nc.gpsimd)[(4 * qk + dt) % 3]
                            else:
                                weng = nc.scalar if qk == 1 else nc.sync
                            weng.dma_start(out=w[:, 0:16, :],
                                           in_=wqkT[qk, dt, :, 0:2048])
                            weng.dma_start(out=w[:, 16:32, :],
                                           in_=wqkT[qk, dt, :, 2048:4096])
                        for h in range(NHT):
                            nc.tensor.matmul(
                                ps[:], w[:, h, :], xsl[h][:],
                                start=(h == 0), stop=(h == NHT - 1))
                        # rope drain -> (qT|kT)[:, dt, s0:s0+SBLK]
                        dst = (qT if qk == 0 else kT)[:, dt, s0:s0 + SBLK]
                        qsb = rpool.tile([128, SBLK], BF16, tag="qsb")
                        nc.scalar.copy(qsb[:], ps[:])
                        qsw = rpool.tile([128, SBLK], BF16, tag="qsw")
                        nc.scalar.dma_start(out=qsw[0:64, :],
                                            in_=qsb[64:128, :])
                        nc.scalar.dma_start(out=qsw[64:128, :],
                                            in_=qsb[0:64, :])
                        t1 = rpool.tile([128, SBLK], BF16, tag="t1")
                        nc.vector.tensor_mul(t1[:], qsb[:],
                                             cos_sb[:, s0:s0 + SBLK])
                        t2 = rpool.tile([128, SBLK], BF16, tag="t2")
                        nc.vector.tensor_mul(t2[:], qsw[:],
                                             sin_sb[:, s0:s0 + SBLK])
                        nc.vector.tensor_add(dst, t1[:], t2[:])
                        pump_po(psD, qeng=nc.scalar, alt=(psB, "b"))

                # V pass: h-grouped so wv streams through a 2-buf ring.
                # psv[st] accumulate over all h; 4 psA banks held at once.
                psv = [psA.tile([128, DPC], F32, tag="a", name=f"pjv{st}")
                       for st in range(SBLK // 128)]
                for g in range(4):
                    wv = wpool.tile([128, 8, DPC], BF16, tag="wv",
                                     bufs=2)
                    nc.gpsimd.dma_start(out=wv[:],
                                        in_=wvT[:, 8 * g:8 * g + 8, :])
                    for hh in range(8):
                        h = 8 * g + hh
                        for st in range(SBLK // 128):
                            nc.tensor.matmul(
                                psv[st][:],
                                xsl[h][:, st * 128:(st + 1) * 128],
                                wv[:, hh, :],
                                start=(h == 0), stop=(h == NHT - 1))
                    pump_po(psD, qeng=nc.scalar, alt=(psB, "b"))
                for st in range(SBLK // 128):
                    nc.vector.tensor_copy(
                        v_sb[:, (s0 + st * 128) // 128, :], psv[st][:])

            # ---------------- attn(b) ----------------
            for qc in range(NQC):
                q0 = qc * QC
                nkt = 4 * qc + 4
                pden = psC.tile([128, QC], F32, tag="c", name="pden")
                pc = []
                # one pass per head (PSUM: 4 pc + 2 pss + 1 den + 1 po).
                # ctx MMs software-pipelined 2 deep behind the score MMs,
                # hiding the mask+exp latency.  Diagonal blocks narrowed
                # to their unmasked q-columns everywhere (scores, exp,
                # ctx); pr is memset-zeroed below c0 only for the quad
                # sum feeding the den MM.
                for h in range(HPC):
                    pch = psA.tile([128, QC], F32, tag="a", name=f"pc{h}")
                    pc.append(pch)
                    pending = []
                    quad = []
                    nq = 0

                    def flush(pch=pch, qc=qc):
                        kt, prt = pending.pop(0)
                        i = kt - 4 * qc
                        c0 = 128 * i if i > 0 else 0
                        nc.tensor.matmul(
                            pch[:, c0:],
                            v_sb[:, kt, h * HD:(h + 1) * HD],
                            prt[:, c0:],
                            start=(kt == 0), stop=(kt == nkt - 1))

                    for kt in range(nkt):
                        i = kt - 4 * qc
                        c0 = 128 * i if i >= 0 else 0
                        pss = psB.tile([128, QC], F32, tag="b", name="pss")
                        nc.tensor.matmul(
                            pss[:, c0:QC],
                            kT[:, h, kt * 128:(kt + 1) * 128],
                            qT[:, h, q0 + c0:q0 + QC],
                            start=True, stop=True)
                        if len(pending) == 4:
                            flush()
                        pr = prpool.tile([128, QC], BF16, tag="pr",
                                         name="pr")
                        nc.scalar.activation(out=pr[:, c0:QC],
                                             in_=pss[:, c0:QC],
                                             func=EXPF)
                        if i >= 0:
                            # zero the masked upper triangle of the diagonal
                            # block (0/1 bf16 mask, off the pss recycle path)
                            nc.vector.tensor_mul(
                                pr[:, c0:c0 + 128], pr[:, c0:c0 + 128],
                                mask_sb[:])
                        pending.append((kt, pr))
                        if i >= 0:
                            # diagonal block: narrowed per-pr den MM (no
                            # zero-fill or quad tree needed)
                            nc.tensor.matmul(
                                pden[:, c0:],
                                id16_sb[:, h, :],
                                pr[:, c0:],
                                start=(h == 0 and qc == 0 and kt == 0),
                                stop=(h == HPC - 1 and kt == nkt - 1))
                        else:
                            quad.append(pr)
                            if len(quad) == 4:
                                # quad-sum (bf16) then one den MM per quad
                                t1q = qpool.tile([128, QC], BF16, tag="t1q", bufs=1)
                                nc.vector.tensor_add(t1q[:], quad[0][:],
                                                     quad[1][:])
                                t2q = qpool.tile([128, QC], BF16, tag="t2q", bufs=1)
                                nc.vector.tensor_add(t2q[:], quad[2][:],
                                                     quad[3][:])
                                t4q = qpool.tile([128, QC], BF16, tag="t4q", bufs=1)
                                nc.vector.tensor_add(t4q[:], t1q[:], t2q[:])
                                nc.tensor.matmul(
                                    pden[:],
                                    id16_sb[:, h, :],
                                    t4q[:],
                                    start=(h == 0 and nq == 0),
                                    stop=False)
                                nq += 1
                                quad = []
                        pump_po(psD, qeng=nc.sync)
                    while pending:
                        flush()
                    pump_po(psD, qeng=nc.sync)
                rec = smpool.tile([HPC, QC], F32, tag="rec", bufs=1)
                nc.vector.reciprocal_approx_fast(out=rec[:], in_=pden[0:HPC, :])
                for h in range(HPC):
                    if h == 0:
                        src_row = rec[0:1, :]
                    else:
                        rh = smpool.tile([1, QC], F32, tag="rh", bufs=1)
                        nc.scalar.dma_start(out=rh[:], in_=rec[h:h + 1, :])
                        src_row = rh[:]
                    rbc = smpool.tile([128, QC], F32, tag="rbc")
                    nc.gpsimd.partition_broadcast(rbc[:], src_row)
                    nc.vector.tensor_mul(ctx_sb[:, h, q0:q0 + QC],
                                         pc[h][:], rbc[:])
                for oc in range(H // 512):
                    for st in range(4 * qc, 4 * qc + 4):
                        pend_po.append((b, oc, st))

            if b == B - 1:
                pump_po(psA, n=len(pend_po), tag="a")

    nc.compile()
    return nc


_CACHE = {}


def _host_prep(x, w_pack, w_o):
    """Build per-core input maps (sharding + layout prep)."""
    x = np.asarray(x, dtype=np.float32)
    w_pack = np.asarray(w_pack, dtype=np.float32)
    w_o = np.asarray(w_o, dtype=np.float32)

    xT = np.ascontiguousarray(
        x.transpose(0, 2, 1).reshape(B, NHT, 128, S)
        .astype(ml_dtypes.bfloat16))                     # [B, 32, 128, S]

    inv_freq = 1.0 / (ROPE_BASE ** (np.arange(0, HD, 2, dtype=np.float32) / HD))
    t = np.arange(S, dtype=np.float32)
    freqs = np.outer(t, inv_freq)                            # [S, HD/2]
    emb = np.concatenate([freqs, freqs], axis=-1)            # [S, HD]
    cosT = np.ascontiguousarray(
        np.cos(emb).T.astype(ml_dtypes.bfloat16))            # [HD, S]
    sinT = np.sin(emb).T.astype(np.float32)
    sinTm = np.concatenate([-sinT[:HD // 2], sinT[HD // 2:]], axis=0)
    sinTm = np.ascontiguousarray(sinTm.astype(ml_dtypes.bfloat16))

    kk2 = np.arange(128)[:, None]
    qq = np.arange(128)[None, :]
    masks = np.ascontiguousarray(
        np.where(kk2 <= qq, 1.0, 0.0).astype(ml_dtypes.bfloat16))  # [128, 128]

    id16 = np.zeros((128, HPC, 128), dtype=np.float32)
    for h in range(HPC):
        id16[:, h, h] = 1.0
    id16 = np.ascontiguousarray(id16.astype(ml_dtypes.bfloat16))

    scale = float(HD) ** -0.5
    in_maps = []
    for c in range(NCORES):
        r0 = c * DPC
        wq = w_pack[r0:r0 + DPC, :] * scale                  # [512, H]
        wk = w_pack[H + r0:H + r0 + DPC, :]
        wv = w_pack[2 * H + r0:2 * H + r0 + DPC, :]
        # wqkT[qk, dt, p, 128h+d] = w^T[128h+p, 128dt+d]
        wqkT = np.stack([wq.T, wk.T], axis=0)                # [2, H, 512]
        wqkT = wqkT.reshape(2, NHT, 128, HPC, 128)           # [2,h,p,dt,d]
        wqkT = wqkT.transpose(0, 3, 2, 1, 4).reshape(2, HPC, 128, NHT * 128)
        wqkT = np.ascontiguousarray(wqkT.astype(ml_dtypes.bfloat16))
        # wvT[p, h, d] = w_v^T[128h+p, d]
        wvT = wv.T.reshape(NHT, 128, DPC).transpose(1, 0, 2)
        wvT = np.ascontiguousarray(wvT.astype(ml_dtypes.bfloat16))
        # woT[p, oc, h, o] = w_o^T[128h+p, 512oc+o]
        woT = w_o[:, r0:r0 + DPC].T.reshape(HPC, 128, H // 512, 512)
        woT = woT.transpose(1, 2, 0, 3)
        woT = np.ascontiguousarray(woT.astype(ml_dtypes.bfloat16))
        in_maps.append({
            "xT": xT, "wqkT": wqkT, "wvT": wvT, "woT": woT,
            "cosT": cosT, "sinTm": sinTm, "id16": id16,
            "masks": masks,
        })
    return in_maps


def kernel(x, w_pack, w_o, _trace=False, _trace_kwargs=None):
    if "nc" not in _CACHE:
        _CACHE["nc"] = _build()
    nc = _CACHE["nc"]

    in_maps = _host_prep(x, w_pack, w_o)
    res = run_bass_kernel_spmd(nc, in_maps, list(range(NCORES)),
                               trace=_trace, **(_trace_kwargs or {}))
    acc = res.results[0]["out"].astype(np.float32)
    for c in range(1, NCORES):
        acc = acc + res.results[c]["out"].astype(np.float32)
    if _trace:
        kernel.last_results = res
    return acc
